# revision 1
# baseline (speedup 1.0000x reference)
"""CrossAttention Trainium2 kernel, v2.

Data-parallel over batch across 8 NeuronCores (4 batches each).

v1 computed attention probabilities P in [query, key] orientation and
transposed them with dma_start_transpose (33.6MB/core of 2-byte-element
XBAR traffic) — that dominated the runtime. v2 computes S^T = K^T·Q in
[key, query] orientation directly, so P^T feeds the AV matmul with no
transpose at all:

  - QK: per 128-token tile, matmul(lhsT=k_tile[hd,128], rhs=qh[hd,128])
    packs the two heads of a pair in PE row-halves (tile_position).
  - softmax: no max-subtraction (logits bounded); exp on ACT engine out
    of PSUM; additive mask folded in multiplicatively (host precomputes
    exp(mask), transposed layout) on DVE.
  - denominators: V gets a ones-column appended (M=65 AV matmuls), so
    row 64 of the AV accumulator is sum_n p — free.
  - normalization: folded into a per-head output projection; denom
    reciprocals land on q-partitions via 8 tiny PE transposes, then one
    fused DVE scalar_tensor_tensor per head does scale+accumulate(+bias).

Engine budget per core (cost model): PE ~335us, ACT ~150us, DVE ~150us,
Pool ~140us, DMA ~60us.
"""
import os
import sys

sys.path.insert(0, "/opt/trn_rl_repo")

VARIANT = os.environ.get("KERNEL_VARIANT", "")
# fp8 (e4m3, DoubleRow) kv projection: 4x fewer PE cycles than bf16.
# "k"  = only K in fp8 (safest), "kv" = both, "" = all-bf16.
FP8 = os.environ.get("KERNEL_FP8", "kv")

import numpy as np
import ml_dtypes

import concourse.bacc as bacc
import concourse.mybir as mybir
import concourse.tile as tile

BF = ml_dtypes.bfloat16

B, QN, N, DIM, HEADS, HD = 32, 128, 4096, 512, 8, 64
SCALE = HD ** -0.5
NCORES = 8
BL = B // NCORES  # batches per core
NT = N // 128     # 32 token tiles
NG = 4            # QK/exp groups per head (8 tiles = 1024 wide each)
GW = N // NG      # group width (psum free bytes: 4KB = 2 banks)

f32 = mybir.dt.float32
bf16 = mybir.dt.bfloat16
fp8 = mybir.dt.float8e4
F8 = ml_dtypes.float8_e4m3fn
DR = mybir.MatmulPerfMode.DoubleRow
W8SCALE = 64.0
MULT = mybir.AluOpType.mult
ADD = mybir.AluOpType.add
EXP = mybir.ActivationFunctionType.Exp
COPY = mybir.ActivationFunctionType.Copy

_built = {}
_runner = {}


def _emit(nc, reps=1):
    if FP8 == "kv":
        kvT_d = nc.dram_tensor(
            "kvT8", [BL, 2, 128, 2, N], fp8, kind="ExternalInput"
        ).ap()
        wkvT_d = nc.dram_tensor(
            "wkv8", [2, 128, 2, 2 * DIM], fp8, kind="ExternalInput"
        ).ap()
    else:
        kvT_d = nc.dram_tensor(
            "kvT", [BL, 4, 128, N], bf16, kind="ExternalInput"
        ).ap()
        wkvT_d = nc.dram_tensor(
            "wkvT", [4, 128, 2 * DIM], bf16, kind="ExternalInput"
        ).ap()
    qT_d = nc.dram_tensor("qT", [4, 128, BL * QN], bf16, kind="ExternalInput").ap()
    emT_d = nc.dram_tensor("emT", [BL, 128, N], bf16, kind="ExternalInput").ap()
    wqT_d = nc.dram_tensor("wqT", [4, 128, DIM], bf16, kind="ExternalInput").ap()
    wpT_d = nc.dram_tensor("wpT", [HEADS, 64, DIM], bf16, kind="ExternalInput").ap()
    bias_d = nc.dram_tensor("biasb", [128, DIM], f32, kind="ExternalInput").ap()
    out_d = nc.dram_tensor("out", [BL, QN, DIM], f32, kind="ExternalOutput").ap()
    dbg = os.environ.get("KERNEL_DEBUG", "") == "1"
    if dbg:
        dbg_kt = nc.dram_tensor("dbg_kt", [4, 128, N], bf16, kind="ExternalOutput").ap()
        dbg_pt = nc.dram_tensor("dbg_pt", [2, 128, N], bf16, kind="ExternalOutput").ap()
        dbg_v = nc.dram_tensor("dbg_v", [2, 128, HEADS, 65], bf16, kind="ExternalOutput").ap()
        dbg_x = nc.dram_tensor("dbg_x", [64, HEADS, 128], bf16, kind="ExternalOutput").ap()
        dbg_d = nc.dram_tensor("dbg_d", [1, HEADS, 128], f32, kind="ExternalOutput").ap()
        dbg_r = nc.dram_tensor("dbg_r", [128, HEADS], f32, kind="ExternalOutput").ap()

    with tile.TileContext(nc) as tc:
        with (
            tc.tile_pool(name="wpool", bufs=1) as wpool,
            tc.tile_pool(name="kvtp", bufs=4) as kvtp,
            tc.tile_pool(name="ktp", bufs=6) as ktp,
            tc.tile_pool(name="vp", bufs=NT) as vp,
            tc.tile_pool(name="emp", bufs=1) as emp,
            tc.tile_pool(name="ptp", bufs=2) as ptp,
            tc.tile_pool(name="xsp", bufs=1) as xsp,
            tc.tile_pool(name="accp", bufs=2) as accp,
            tc.tile_pool(name="mm512", bufs=2, space="PSUM") as mm512,
            tc.tile_pool(name="qkps", bufs=1, space="PSUM") as qkps,
            tc.tile_pool(name="xaps", bufs=1, space="PSUM") as xaps,
        ):
            # ---- persistent weights ----
            wkvT, wqT, wpT, qT = [], [], [], []
            for t in range(2 if FP8 == "kv" else 4):
                if FP8 == "kv":
                    wk = wpool.tile([128, 2, 2 * DIM], fp8, name=f"wkvT{t}")
                else:
                    wk = wpool.tile([128, 2 * DIM], bf16, name=f"wkvT{t}")
                nc.sync.dma_start(out=wk, in_=wkvT_d[t])
                wkvT.append(wk)
            for t in range(4):
                wq = wpool.tile([128, DIM], bf16, name=f"wqT{t}")
                nc.sync.dma_start(out=wq, in_=wqT_d[t])
                wqT.append(wq)
                qt = wpool.tile([128, BL * QN], bf16, name=f"qT{t}")
                nc.sync.dma_start(out=qt, in_=qT_d[t])
                qT.append(qt)
            for h in range(HEADS):
                wp = wpool.tile([64, DIM], bf16, name=f"wpT{h}")
                nc.sync.dma_start(out=wp, in_=wpT_d[h])
                wpT.append(wp)
            bias_sb = wpool.tile([128, DIM], f32, name="bias_sb")
            nc.sync.dma_start(out=bias_sb, in_=bias_d)
            ident1 = wpool.tile([1, 1], f32, name="ident1")
            nc.vector.memset(ident1, 1.0)

            # ---- q projection for all local batches: qhT[co] = [c 128, (b q) 512]
            qhT = []
            for co in range(4):
                ps_q = mm512.tile([128, BL * QN], f32, name="ps_mm512")
                for ci in range(4):
                    nc.tensor.matmul(
                        ps_q,
                        wqT[ci][:, co * 128:(co + 1) * 128],
                        qT[ci],
                        start=(ci == 0),
                        stop=(ci == 3),
                    )
                qh = wpool.tile([128, BL * QN], bf16, name=f"qhT{co}")
                nc.vector.tensor_copy(qh, ps_q)
                qhT.append(qh)

            def fetch_kv(b):
                kvt = []
                if FP8 == "kv":
                    for t in range(2):
                        kv_t = kvtp.tile([128, 2, N], fp8, name="kv_t")
                        nc.sync.dma_start(out=kv_t, in_=kvT_d[b, t])
                        kvt.append(kv_t)
                else:
                    for t in range(4):
                        kv_t = kvtp.tile([128, N], bf16, name="kv_t")
                        nc.sync.dma_start(out=kv_t, in_=kvT_d[b, t])
                        kvt.append(kv_t)
                return kvt

            def fetch_em(b):
                em_t = emp.tile([128, N], bf16, name="em_t")
                nc.sync.dma_start(out=em_t, in_=emT_d[b])
                return em_t

            def proj_denoms(xaug):
                """Copy X^T + denoms out of PSUM; reciprocals on q-partitions."""
                x_sb = xsp.tile([64, HEADS, 128], bf16, name="x_sb")
                nc.vector.tensor_copy(x_sb, xaug[0:64])
                d_sb = xsp.tile([1, HEADS, 128], f32, name="d_sb")
                nc.vector.tensor_copy(d_sb, xaug[64:65])
                dT = mm512.tile([128, 512], f32, name="ps_mm512")
                for h in range(HEADS):
                    nc.tensor.matmul(
                        dT[:, h:h + 1],
                        d_sb[:, h, :],
                        ident1,
                        is_transpose=True,
                        start=True,
                        stop=True,
                    )
                dtp_sb = xsp.tile([128, HEADS], f32, name="dtp_sb")
                nc.vector.tensor_copy(dtp_sb, dT[:, 0:HEADS])
                recips = xsp.tile([128, HEADS], f32, name="recips")
                nc.vector.reciprocal(recips, dtp_sb)
                return x_sb, recips

            def proj_head(h, x_sb, recips, acc):
                """One head of output projection + fused normalize-accumulate."""
                ps = mm512.tile([128, DIM], f32, name="ps_mm512")
                nc.tensor.matmul(
                    ps,
                    x_sb[:, h, :],
                    wpT[h],
                    start=True,
                    stop=True,
                )
                nc.vector.scalar_tensor_tensor(
                    out=acc,
                    in0=ps,
                    scalar=recips[:, h:h + 1],
                    in1=(bias_sb if h == 0 else acc),
                    op0=MULT,
                    op1=ADD,
                )

            def kquad(kvt, kt, ko, ch):
                ps = mm512.tile([128, 512], f32, name="ps_mm512")
                if FP8 == "kv":
                    for hf in range(2):
                        nc.tensor.matmul(
                            ps,
                            wkvT[hf][:, :, ko * 128:(ko + 1) * 128],
                            kvt[hf][:, :, ch * 512:(ch + 1) * 512],
                            start=(hf == 0),
                            stop=(hf == 1),
                            perf_mode=DR,
                        )
                    nc.vector.tensor_scalar_mul(
                        kt[ko][:, ch * 512:(ch + 1) * 512], ps, 1.0 / W8SCALE
                    )
                else:
                    for ci in range(4):
                        nc.tensor.matmul(
                            ps,
                            wkvT[ci][:, ko * 128:(ko + 1) * 128],
                            kvt[ci][:, ch * 512:(ch + 1) * 512],
                            start=(ci == 0),
                            stop=(ci == 3),
                        )
                    nc.vector.tensor_copy(kt[ko][:, ch * 512:(ch + 1) * 512], ps)

            def vquad(kvt, vt, tt):
                ps = mm512.tile([128, 512], f32, name="ps_mm512")
                if FP8 == "kv":
                    for hf in range(2):
                        nc.tensor.matmul(
                            ps,
                            kvt[hf][:, :, tt * 128:(tt + 1) * 128],
                            wkvT[hf][:, :, DIM:2 * DIM],
                            start=(hf == 0),
                            stop=(hf == 1),
                            perf_mode=DR,
                        )
                    scale = 1.0 / W8SCALE
                else:
                    for ci in range(4):
                        nc.tensor.matmul(
                            ps,
                            kvt[ci][:, tt * 128:(tt + 1) * 128],
                            wkvT[ci][:, DIM:2 * DIM],
                            start=(ci == 0),
                            stop=(ci == 3),
                        )
                    scale = 1.0
                psr = ps[:, :].rearrange("p (h d) -> p h d", h=HEADS)
                if tt % 2 == 0:
                    nc.scalar.activation(vt[tt][:, :, 0:64], psr, COPY, scale=scale)
                else:
                    nc.vector.tensor_scalar_mul(vt[tt][:, :, 0:64], psr, scale)
                nc.gpsimd.memset(vt[tt][:, :, 64:65], 1.0)

            def emit_av_chunk(xaug, vt, h, pt, ts):
                # Serial accumulation chain per head: interleaved open chains
                # in one PSUM bank corrupt the accumulator, so a head's chain
                # only interleaves with matmuls targeting OTHER banks.
                for t in ts:
                    nc.tensor.matmul(
                        xaug[:, h, :],
                        vt[t][:, h, :],
                        pt[:, t * 128:(t + 1) * 128],
                        start=(t == 0),
                        stop=(t == NT - 1),
                        skip_group_check=True,
                    )

            def emit_av_pair(xaug, vt, pr, pt0, pt1):
                for h, pt in ((2 * pr, pt0), (2 * pr + 1, pt1)):
                    emit_av_chunk(xaug, vt, h, pt, range(NT))

            steps = [b for _ in range(reps) for b in range(BL)]
            kvt = fetch_kv(steps[0])
            em_t = fetch_em(steps[0])
            av_carry = None  # (xaug, vt, pt0, pt1, b) from previous step
            pending = None   # (b, x_sb, recips) awaiting proj phase 2
            kt_next = None   # kt tiles [ko0, ko1] prefilled during prev B

            for i, b in enumerate(steps):
                last = i + 1 == len(steps)

                # ---- A phase: finish prev batch (AV p3 + denoms + proj) and
                # ---- run this batch's remaining kv projection.
                if av_carry is not None:
                    xaug_p, vt_p, pt0_p, pt1_p, b_p = av_carry
                    emit_av_pair(xaug_p, vt_p, 3, pt0_p, pt1_p)
                    x_sb, recips = proj_denoms(xaug_p)
                    if dbg and i == 1:
                        nc.sync.dma_start(out=dbg_x, in_=x_sb)
                        nc.sync.dma_start(out=dbg_r, in_=recips)
                    pending = (b_p, x_sb, recips)
                    av_carry = None
                if kt_next is None:
                    kt = [ktp.tile([128, N], bf16, name="k_t") for _ in range(4)]
                    quads = [("k", ko, ch) for ko in (0, 1) for ch in range(8)]
                else:
                    kt = kt_next + [
                        ktp.tile([128, N], bf16, name="k_t") for _ in range(2)
                    ]
                    quads = []
                quads += [("k", ko, ch) for ko in (2, 3) for ch in range(8)]
                vt = [vp.tile([128, HEADS, 65], bf16, name="v_t") for _ in range(NT)]
                quads += [("v", tt, 0) for tt in range(NT)]
                acc = None
                for qi, (kind, a0, a1) in enumerate(quads):
                    if pending is not None and 4 <= qi < 4 + HEADS:
                        if qi == 4:
                            acc = accp.tile([128, DIM], f32, name="acc")
                        proj_head(qi - 4, pending[1], pending[2], acc)
                    if kind == "k":
                        kquad(kvt, kt, a0, a1)
                    else:
                        vquad(kvt, vt, a0)
                    if pending is not None and qi == 4 + HEADS:
                        nc.sync.dma_start(out=out_d[pending[0]], in_=acc)
                        pending = None

                # ---- B phase: attention; prefill next batch's kvt + kt(0,1)
                # ---- in the ACT-paced slot gaps.
                if not last:
                    kv_next = fetch_kv(steps[i + 1])
                    kt_next = [
                        ktp.tile([128, N], bf16, name="k_t") for _ in range(2)
                    ]
                    kq = [(ko, ch) for ko in (0, 1) for ch in range(8)]
                else:
                    kv_next, kt_next, kq = None, None, []
                xaug = xaps.tile([65, HEADS, 128], f32, name="xaug")
                prev = None  # (pr, pt0, pt1)
                slot = 0
                for pr in range(4):
                    pt0 = ptp.tile([128, N], bf16, name="pt0")
                    pt1 = ptp.tile([128, N], bf16, name="pt1")
                    for g in range(NG):
                        ps0 = qkps.tile([128, GW], f32, name="ps_s0")
                        ps1 = qkps.tile([128, GW], f32, name="ps_s1")
                        for j in range(GW // 128):
                            t = (GW // 128) * g + j
                            nc.tensor.matmul(
                                ps0[:, j * 128:(j + 1) * 128],
                                kt[pr][0:64, t * 128:(t + 1) * 128],
                                qhT[pr][0:64, b * QN:(b + 1) * QN],
                                start=True,
                                stop=True,
                                tile_position=(0, 0),
                            )
                            nc.tensor.matmul(
                                ps1[:, j * 128:(j + 1) * 128],
                                kt[pr][64:128, t * 128:(t + 1) * 128],
                                qhT[pr][64:128, b * QN:(b + 1) * QN],
                                start=True,
                                stop=True,
                                tile_position=(64, 0),
                            )
                        sl = slice(g * GW, (g + 1) * GW)
                        nc.scalar.activation(pt0[:, sl], ps0, EXP)
                        nc.scalar.activation(pt1[:, sl], ps1, EXP)
                        nc.vector.tensor_mul(pt0[:, sl], pt0[:, sl], em_t[:, sl])
                        nc.vector.tensor_mul(pt1[:, sl], pt1[:, sl], em_t[:, sl])
                        if pr >= 1:
                            # AV quarter of pair pr-1 (serial per head)
                            h = 2 * (pr - 1) + g // 2
                            pt = prev[1] if g < 2 else prev[2]
                            t0 = (g % 2) * (NT // 2)
                            emit_av_chunk(xaug, vt, h, pt, range(t0, t0 + NT // 2))
                            for _ in range(2 if slot >= 8 else 1):
                                if kq:
                                    ko, ch = kq.pop(0)
                                    kquad(kv_next, kt_next, ko, ch)
                            slot += 1
                    if dbg and i == 0 and pr == 0:
                        nc.sync.dma_start(out=dbg_pt[0], in_=pt0)
                        nc.sync.dma_start(out=dbg_pt[1], in_=pt1)
                    prev = (pr, pt0, pt1)
                if dbg and i == 0:
                    for t in range(4):
                        nc.sync.dma_start(out=dbg_kt[t], in_=kt[t])
                    for t in range(2):
                        nc.sync.dma_start(out=dbg_v[t], in_=vt[t])
                av_carry = (xaug, vt, prev[1], prev[2], b)
                if not last:
                    kvt = kv_next
                    em_t = fetch_em(steps[i + 1])

            # epilogue: flush last batch
            xaug_p, vt_p, pt0_p, pt1_p, b_p = av_carry
            emit_av_pair(xaug_p, vt_p, 3, pt0_p, pt1_p)
            x_sb, recips = proj_denoms(xaug_p)
            acc = accp.tile([128, DIM], f32, name="acc")
            for h in range(HEADS):
                proj_head(h, x_sb, recips, acc)
            nc.sync.dma_start(out=out_d[b_p], in_=acc)
    return nc


def build(reps=1):
    if reps not in _built:
        nc = bacc.Bacc(
            "TRN2", target_bir_lowering=False, debug=False, num_devices=NCORES
        )
        _emit(nc, reps)
        nc.compile()
        _built[reps] = nc
    return _built[reps]


def prep_inputs(q, kv, key_mask, Wq, Wkv, Wproj, bproj):
    """Host-side shard + layout prep. Returns per-core in_maps."""
    q = np.asarray(q, dtype=np.float32)
    kv = np.asarray(kv, dtype=np.float32)
    key_mask = np.asarray(key_mask, dtype=np.float32)
    wkvT_f = np.ascontiguousarray(np.asarray(Wkv, np.float32).T)  # [512, 1024]
    if FP8 == "kv":
        # c = (half*2 + i)*128 + k  ->  wkv8[half][k, i, :]
        wkv8 = (wkvT_f * W8SCALE).reshape(2, 2, 128, 2 * DIM)
        wkv8 = np.ascontiguousarray(wkv8.transpose(0, 2, 1, 3)).astype(F8)
    else:
        wkvT = wkvT_f.astype(BF).reshape(4, 128, 2 * DIM)
    wqT = np.ascontiguousarray((np.asarray(Wq, np.float32) * SCALE).T).astype(BF)
    wqT = wqT.reshape(4, 128, DIM)
    wpT = np.ascontiguousarray(np.asarray(Wproj, np.float32).T).astype(BF)
    wpT = wpT.reshape(HEADS, 64, DIM)
    biasb = np.ascontiguousarray(
        np.broadcast_to(np.asarray(bproj, np.float32), (128, DIM))
    )

    kv_bf = kv.astype(BF)
    em = np.exp(key_mask).astype(BF)  # [B, QN, N]

    in_maps = []
    for c in range(NCORES):
        sl = slice(c * BL, (c + 1) * BL)
        q_loc = q[sl].astype(BF)  # [BL, QN, DIM]
        qT = np.ascontiguousarray(q_loc.transpose(2, 0, 1)).reshape(4, 128, BL * QN)
        # emT[b, n_in_tile, tt*128 + q] = em[b, q, tt*128 + n_in_tile]
        emT = em[sl].reshape(BL, QN, NT, 128).transpose(0, 3, 2, 1)
        emT = np.ascontiguousarray(emT).reshape(BL, 128, N)
        m = {"qT": qT, "emT": emT, "wqT": wqT, "wpT": wpT, "biasb": biasb}
        if FP8 == "kv":
            kvc = kv[sl].transpose(0, 2, 1)  # [BL, 512, N] f32
            kv8 = kvc.reshape(BL, 2, 2, 128, N).transpose(0, 1, 3, 2, 4)
            m["kvT8"] = np.ascontiguousarray(kv8).astype(F8)
            m["wkv8"] = wkv8
        else:
            m["kvT"] = np.ascontiguousarray(kv_bf[sl].transpose(0, 2, 1)).reshape(
                BL, 4, 128, N
            )
            m["wkvT"] = wkvT
        in_maps.append(m)
    return in_maps


class Runner:
    """Jitted SPMD executor with device-resident inputs for repeat timing."""

    def __init__(self, reps=1):
        import jax
        from concourse.bass2jax import (
            _bass_exec_p,
            install_neuronx_cc_hook,
            partition_id_tensor,
        )
        from jax.experimental.shard_map import shard_map
        from jax.sharding import Mesh, PartitionSpec

        self.jax = jax
        nc = build(reps)
        install_neuronx_cc_hook()
        pname = nc.partition_id_tensor.name if nc.partition_id_tensor else None
        in_names, out_names, out_avals = [], [], []
        for alloc in nc.m.functions[0].allocations:
            if not isinstance(alloc, mybir.MemoryLocationSet):
                continue
            name = alloc.memorylocations[0].name
            if alloc.kind == "ExternalInput":
                if name != pname:
                    in_names.append(name)
            elif alloc.kind == "ExternalOutput":
                out_names.append(name)
                out_avals.append(
                    jax.core.ShapedArray(
                        tuple(alloc.tensor_shape), mybir.dt.np(alloc.dtype)
                    )
                )
        self.in_names = list(in_names)
        self.out_names = out_names
        self.out_avals = out_avals
        n_params = len(in_names)
        all_names = in_names + out_names
        if pname is not None:
            all_names = all_names + [pname]
        donate = tuple(range(n_params, n_params + len(out_names)))

        def _body(*args):
            operands = list(args)
            if pname is not None:
                operands.append(partition_id_tensor())
            outs = _bass_exec_p.bind(
                *operands,
                out_avals=tuple(out_avals),
                in_names=tuple(all_names),
                out_names=tuple(out_names),
                lowering_input_output_aliases=(),
                sim_require_finite=True,
                sim_require_nnan=True,
                nc=nc,
            )
            return tuple(outs)

        devices = jax.devices()[:NCORES]
        self.mesh = Mesh(np.asarray(devices), ("core",))
        self.pspec = PartitionSpec("core")
        in_specs = (self.pspec,) * (n_params + len(out_names))
        out_specs = (self.pspec,) * len(out_names)
        self.fn = jax.jit(
            shard_map(
                _body,
                mesh=self.mesh,
                in_specs=in_specs,
                out_specs=out_specs,
                check_rep=False,
            ),
            donate_argnums=donate,
            keep_unused=True,
        )

    def put_inputs(self, in_maps):
        """Concat per-core inputs on axis 0 and move to devices (sharded)."""
        from jax.sharding import NamedSharding

        sh = NamedSharding(self.mesh, self.pspec)
        dev = []
        for name in self.in_names:
            cat = np.concatenate([m[name] for m in in_maps], axis=0)
            dev.append(self.jax.device_put(cat, sh))
        return dev

    def zeros(self):
        from jax.sharding import NamedSharding

        sh = NamedSharding(self.mesh, self.pspec)
        return [
            self.jax.device_put(
                np.zeros((NCORES * a.shape[0], *a.shape[1:]), a.dtype), sh
            )
            for a in self.out_avals
        ]

    def run(self, dev_inputs, zeros=None):
        if zeros is None:
            zeros = self.zeros()
        outs = self.fn(*dev_inputs, *zeros)
        self.jax.block_until_ready(outs)
        return outs


def get_runner(reps=1):
    if reps not in _runner:
        _runner[reps] = Runner(reps)
    return _runner[reps]


def kernel(q, kv, key_mask, Wq, Wkv, Wproj, bproj):
    r = get_runner()
    in_maps = prep_inputs(q, kv, key_mask, Wq, Wkv, Wproj, bproj)
    dev = r.put_inputs(in_maps)
    outs = r.run(dev)
    out = np.asarray(outs[0]).reshape(NCORES, BL, QN, DIM).reshape(B, QN, DIM)
    return out.astype(np.float32)



# revision 2
# speedup vs baseline: 10.0898x; 10.0898x over previous
"""CrossAttention Trainium2 kernel, v3.

Data-parallel over batch across 8 NeuronCores (4 batches each).

v1 computed attention probabilities P in [query, key] orientation and
transposed them with dma_start_transpose (33.6MB/core of 2-byte-element
XBAR traffic) — that dominated the runtime. v2 computes S^T = K^T·Q in
[key, query] orientation directly, so P^T feeds the AV matmul with no
transpose at all:

  - QK: per 128-token tile, matmul(lhsT=k_tile[hd,128], rhs=qh[hd,128])
    packs the two heads of a pair in PE row-halves (tile_position).
  - softmax: no max-subtraction (logits bounded); exp on ACT engine out
    of PSUM; additive mask folded in multiplicatively (host precomputes
    exp(mask), transposed layout) on DVE.
  - denominators: V gets a ones-column appended (M=65 AV matmuls), so
    row 64 of the AV accumulator is sum_n p — free.
  - normalization: folded into a per-head output projection; denom
    reciprocals land on q-partitions via 8 tiny PE transposes, then one
    fused DVE scalar_tensor_tensor per head does scale+accumulate(+bias).

v3 changes (HW-attribution-driven; knockouts showed the kv-projection
pipeline cost 183us of the 387us/rep device time, dominated by 1x-mode
PSUM->SBUF copies piled onto DVE while ACT idled in phase A):
  - kquad PSUM->SBUF copies alternate DVE/ACT (was all-DVE).
  - next batch's V projection (VPREF tiles) + K ko0/1 interleave into
    phase B's ACT-bound slot gaps, shrinking the serial phase A.
  - em (mask) tiles double-buffered so the next batch's DMA overlaps.
  - Runner uses fast_dispatch_compile (C++ fast-path dispatch): ~300us
    less per-exec overhead and executions pipeline on the device queue.
Measured per-rep device time (reps-slope, 4 batches/core): 387us -> 343us.
"""
import os
import sys

sys.path.insert(0, "/opt/trn_rl_repo")

VARIANT = os.environ.get("KERNEL_VARIANT", "")
# fp8 (e4m3, DoubleRow) kv projection: 4x fewer PE cycles than bf16.
# "k"  = only K in fp8 (safest), "kv" = both, "" = all-bf16.
FP8 = os.environ.get("KERNEL_FP8", "kv")
# comma-separated phase knockouts for attribution experiments:
# emmul, exp, av, qk, kq, vq
SKIP = set(filter(None, os.environ.get("KERNEL_SKIP", "").split(",")))

import numpy as np
import ml_dtypes

import concourse.bacc as bacc
import concourse.mybir as mybir
import concourse.tile as tile

BF = ml_dtypes.bfloat16

B, QN, N, DIM, HEADS, HD = 32, 128, 4096, 512, 8, 64
VPREF = 10  # next-batch V tiles projected during phase B
SCALE = HD ** -0.5
NCORES = 8
BL = B // NCORES  # batches per core
NT = N // 128     # 32 token tiles
NG = 4            # QK/exp groups per head (8 tiles = 1024 wide each)
GW = N // NG      # group width (psum free bytes: 4KB = 2 banks)

f32 = mybir.dt.float32
bf16 = mybir.dt.bfloat16
fp8 = mybir.dt.float8e4
F8 = ml_dtypes.float8_e4m3fn
DR = mybir.MatmulPerfMode.DoubleRow
W8SCALE = 64.0
MULT = mybir.AluOpType.mult
ADD = mybir.AluOpType.add
EXP = mybir.ActivationFunctionType.Exp
COPY = mybir.ActivationFunctionType.Copy

_built = {}
_runner = {}


def _emit(nc, reps=1):
    if FP8 == "kv":
        kvT_d = nc.dram_tensor(
            "kvT8", [BL, 2, 128, 2, N], fp8, kind="ExternalInput"
        ).ap()
        wkvT_d = nc.dram_tensor(
            "wkv8", [2, 128, 2, 2 * DIM], fp8, kind="ExternalInput"
        ).ap()
    else:
        kvT_d = nc.dram_tensor(
            "kvT", [BL, 4, 128, N], bf16, kind="ExternalInput"
        ).ap()
        wkvT_d = nc.dram_tensor(
            "wkvT", [4, 128, 2 * DIM], bf16, kind="ExternalInput"
        ).ap()
    qT_d = nc.dram_tensor("qT", [4, 128, BL * QN], bf16, kind="ExternalInput").ap()
    emT_d = nc.dram_tensor("emT", [BL, 128, N], bf16, kind="ExternalInput").ap()
    wqT_d = nc.dram_tensor("wqT", [4, 128, DIM], bf16, kind="ExternalInput").ap()
    wpT_d = nc.dram_tensor("wpT", [HEADS, 64, DIM], bf16, kind="ExternalInput").ap()
    bias_d = nc.dram_tensor("biasb", [128, DIM], f32, kind="ExternalInput").ap()
    out_d = nc.dram_tensor("out", [BL, QN, DIM], f32, kind="ExternalOutput").ap()
    dbg = os.environ.get("KERNEL_DEBUG", "") == "1"
    if dbg:
        dbg_kt = nc.dram_tensor("dbg_kt", [4, 128, N], bf16, kind="ExternalOutput").ap()
        dbg_pt = nc.dram_tensor("dbg_pt", [2, 128, N], bf16, kind="ExternalOutput").ap()
        dbg_v = nc.dram_tensor("dbg_v", [2, 128, HEADS, 65], bf16, kind="ExternalOutput").ap()
        dbg_x = nc.dram_tensor("dbg_x", [64, HEADS, 128], bf16, kind="ExternalOutput").ap()
        dbg_d = nc.dram_tensor("dbg_d", [1, HEADS, 128], f32, kind="ExternalOutput").ap()
        dbg_r = nc.dram_tensor("dbg_r", [128, HEADS], f32, kind="ExternalOutput").ap()

    with tile.TileContext(nc) as tc:
        with (
            tc.tile_pool(name="wpool", bufs=1) as wpool,
            tc.tile_pool(name="kvtp", bufs=4) as kvtp,
            tc.tile_pool(name="ktp", bufs=6) as ktp,
            tc.tile_pool(name="vp", bufs=NT + VPREF) as vp,
            tc.tile_pool(name="emp", bufs=2) as emp,
            tc.tile_pool(name="ptp", bufs=2) as ptp,
            tc.tile_pool(name="xsp", bufs=1) as xsp,
            tc.tile_pool(name="accp", bufs=2) as accp,
            tc.tile_pool(name="mm512", bufs=2, space="PSUM") as mm512,
            tc.tile_pool(name="qkps", bufs=1, space="PSUM") as qkps,
            tc.tile_pool(name="xaps", bufs=1, space="PSUM") as xaps,
        ):
            # ---- persistent weights ----
            wkvT, wqT, wpT, qT = [], [], [], []
            for t in range(2 if FP8 == "kv" else 4):
                if FP8 == "kv":
                    wk = wpool.tile([128, 2, 2 * DIM], fp8, name=f"wkvT{t}")
                else:
                    wk = wpool.tile([128, 2 * DIM], bf16, name=f"wkvT{t}")
                nc.sync.dma_start(out=wk, in_=wkvT_d[t])
                wkvT.append(wk)
            for t in range(4):
                wq = wpool.tile([128, DIM], bf16, name=f"wqT{t}")
                nc.sync.dma_start(out=wq, in_=wqT_d[t])
                wqT.append(wq)
                qt = wpool.tile([128, BL * QN], bf16, name=f"qT{t}")
                nc.sync.dma_start(out=qt, in_=qT_d[t])
                qT.append(qt)
            for h in range(HEADS):
                wp = wpool.tile([64, DIM], bf16, name=f"wpT{h}")
                nc.sync.dma_start(out=wp, in_=wpT_d[h])
                wpT.append(wp)
            bias_sb = wpool.tile([128, DIM], f32, name="bias_sb")
            nc.sync.dma_start(out=bias_sb, in_=bias_d)
            ident1 = wpool.tile([1, 1], f32, name="ident1")
            nc.vector.memset(ident1, 1.0)

            # ---- q projection for all local batches: qhT[co] = [c 128, (b q) 512]
            qhT = []
            for co in range(4):
                ps_q = mm512.tile([128, BL * QN], f32, name="ps_mm512")
                for ci in range(4):
                    nc.tensor.matmul(
                        ps_q,
                        wqT[ci][:, co * 128:(co + 1) * 128],
                        qT[ci],
                        start=(ci == 0),
                        stop=(ci == 3),
                    )
                qh = wpool.tile([128, BL * QN], bf16, name=f"qhT{co}")
                nc.vector.tensor_copy(qh, ps_q)
                qhT.append(qh)

            def fetch_kv(b):
                kvt = []
                if FP8 == "kv":
                    for t in range(2):
                        kv_t = kvtp.tile([128, 2, N], fp8, name="kv_t")
                        nc.sync.dma_start(out=kv_t, in_=kvT_d[b, t])
                        kvt.append(kv_t)
                else:
                    for t in range(4):
                        kv_t = kvtp.tile([128, N], bf16, name="kv_t")
                        nc.sync.dma_start(out=kv_t, in_=kvT_d[b, t])
                        kvt.append(kv_t)
                return kvt

            def fetch_em(b):
                em_t = emp.tile([128, N], bf16, name="em_t")
                nc.sync.dma_start(out=em_t, in_=emT_d[b])
                return em_t

            def proj_denoms(xaug):
                """Copy X^T + denoms out of PSUM; reciprocals on q-partitions."""
                x_sb = xsp.tile([64, HEADS, 128], bf16, name="x_sb")
                nc.vector.tensor_copy(x_sb, xaug[0:64])
                d_sb = xsp.tile([1, HEADS, 128], f32, name="d_sb")
                nc.vector.tensor_copy(d_sb, xaug[64:65])
                dT = mm512.tile([128, 512], f32, name="ps_mm512")
                for h in range(HEADS):
                    nc.tensor.matmul(
                        dT[:, h:h + 1],
                        d_sb[:, h, :],
                        ident1,
                        is_transpose=True,
                        start=True,
                        stop=True,
                    )
                dtp_sb = xsp.tile([128, HEADS], f32, name="dtp_sb")
                nc.vector.tensor_copy(dtp_sb, dT[:, 0:HEADS])
                recips = xsp.tile([128, HEADS], f32, name="recips")
                nc.vector.reciprocal(recips, dtp_sb)
                return x_sb, recips

            def proj_head(h, x_sb, recips, acc):
                """One head of output projection + fused normalize-accumulate."""
                ps = mm512.tile([128, DIM], f32, name="ps_mm512")
                nc.tensor.matmul(
                    ps,
                    x_sb[:, h, :],
                    wpT[h],
                    start=True,
                    stop=True,
                )
                nc.vector.scalar_tensor_tensor(
                    out=acc,
                    in0=ps,
                    scalar=recips[:, h:h + 1],
                    in1=(bias_sb if h == 0 else acc),
                    op0=MULT,
                    op1=ADD,
                )

            def kquad(kvt, kt, ko, ch):
                if "kq" in SKIP:
                    nc.gpsimd.memset(kt[ko][:, ch * 512:ch * 512 + 16], 0.0)
                    return
                ps = mm512.tile([128, 512], f32, name="ps_mm512")
                if FP8 == "kv":
                    for hf in range(2):
                        nc.tensor.matmul(
                            ps,
                            wkvT[hf][:, :, ko * 128:(ko + 1) * 128],
                            kvt[hf][:, :, ch * 512:(ch + 1) * 512],
                            start=(hf == 0),
                            stop=(hf == 1),
                            perf_mode=DR,
                        )
                    if (ko + ch) % 2 == 0:
                        nc.scalar.activation(
                            kt[ko][:, ch * 512:(ch + 1) * 512], ps, COPY,
                            scale=1.0 / W8SCALE,
                        )
                    else:
                        nc.vector.tensor_scalar_mul(
                            kt[ko][:, ch * 512:(ch + 1) * 512], ps, 1.0 / W8SCALE
                        )
                else:
                    for ci in range(4):
                        nc.tensor.matmul(
                            ps,
                            wkvT[ci][:, ko * 128:(ko + 1) * 128],
                            kvt[ci][:, ch * 512:(ch + 1) * 512],
                            start=(ci == 0),
                            stop=(ci == 3),
                        )
                    nc.vector.tensor_copy(kt[ko][:, ch * 512:(ch + 1) * 512], ps)

            def vquad(kvt, vt, tt):
                if "vq" in SKIP:
                    nc.gpsimd.memset(vt[tt][:, :, 0:2], 0.0)
                    nc.gpsimd.memset(vt[tt][:, :, 64:65], 1.0)
                    return
                ps = mm512.tile([128, 512], f32, name="ps_mm512")
                if FP8 == "kv":
                    for hf in range(2):
                        nc.tensor.matmul(
                            ps,
                            kvt[hf][:, :, tt * 128:(tt + 1) * 128],
                            wkvT[hf][:, :, DIM:2 * DIM],
                            start=(hf == 0),
                            stop=(hf == 1),
                            perf_mode=DR,
                        )
                    scale = 1.0 / W8SCALE
                else:
                    for ci in range(4):
                        nc.tensor.matmul(
                            ps,
                            kvt[ci][:, tt * 128:(tt + 1) * 128],
                            wkvT[ci][:, DIM:2 * DIM],
                            start=(ci == 0),
                            stop=(ci == 3),
                        )
                    scale = 1.0
                psr = ps[:, :].rearrange("p (h d) -> p h d", h=HEADS)
                if tt % 2 == 0:
                    nc.scalar.activation(vt[tt][:, :, 0:64], psr, COPY, scale=scale)
                else:
                    nc.vector.tensor_scalar_mul(vt[tt][:, :, 0:64], psr, scale)
                nc.gpsimd.memset(vt[tt][:, :, 64:65], 1.0)

            def emit_av_chunk(xaug, vt, h, pt, ts):
                # Serial accumulation chain per head: interleaved open chains
                # in one PSUM bank corrupt the accumulator, so a head's chain
                # only interleaves with matmuls targeting OTHER banks.
                if "av" in SKIP:
                    if ts and ts[0] == 0:
                        nc.tensor.matmul(
                            xaug[:, h, :],
                            vt[0][:, h, :],
                            pt[:, 0:128],
                            start=True,
                            stop=True,
                            skip_group_check=True,
                        )
                    return
                for t in ts:
                    nc.tensor.matmul(
                        xaug[:, h, :],
                        vt[t][:, h, :],
                        pt[:, t * 128:(t + 1) * 128],
                        start=(t == 0),
                        stop=(t == NT - 1),
                        skip_group_check=True,
                    )

            def emit_av_pair(xaug, vt, pr, pt0, pt1):
                for h, pt in ((2 * pr, pt0), (2 * pr + 1, pt1)):
                    emit_av_chunk(xaug, vt, h, pt, range(NT))

            steps = [b for _ in range(reps) for b in range(BL)]
            kvt = fetch_kv(steps[0])
            em_t = fetch_em(steps[0])
            av_carry = None  # (xaug, vt, pt0, pt1, b) from previous step
            pending = None   # (b, x_sb, recips) awaiting proj phase 2
            kt_next = None   # kt tiles [ko0, ko1] prefilled during prev B
            vt_next = None   # vt tiles [0..NT//2) prefilled during prev B

            for i, b in enumerate(steps):
                last = i + 1 == len(steps)

                # ---- A phase: finish prev batch (AV p3 + denoms + proj) and
                # ---- run this batch's remaining kv projection.
                if av_carry is not None:
                    xaug_p, vt_p, pt0_p, pt1_p, b_p = av_carry
                    emit_av_pair(xaug_p, vt_p, 3, pt0_p, pt1_p)
                    x_sb, recips = proj_denoms(xaug_p)
                    if dbg and i == 1:
                        nc.sync.dma_start(out=dbg_x, in_=x_sb)
                        nc.sync.dma_start(out=dbg_r, in_=recips)
                    pending = (b_p, x_sb, recips)
                    av_carry = None
                if kt_next is None:
                    kt = [ktp.tile([128, N], bf16, name="k_t") for _ in range(4)]
                    quads = [("k", ko, ch) for ko in (0, 1) for ch in range(8)]
                else:
                    kt = kt_next + [
                        ktp.tile([128, N], bf16, name="k_t") for _ in range(2)
                    ]
                    quads = []
                quads += [("k", ko, ch) for ko in (2, 3) for ch in range(8)]
                if vt_next is None:
                    vt = [vp.tile([128, HEADS, 65], bf16, name="v_t") for _ in range(NT)]
                    quads += [("v", tt, 0) for tt in range(NT)]
                else:
                    vt = vt_next + [
                        vp.tile([128, HEADS, 65], bf16, name="v_t")
                        for _ in range(NT - VPREF)
                    ]
                    quads += [("v", tt, 0) for tt in range(VPREF, NT)]
                acc = None
                for qi, (kind, a0, a1) in enumerate(quads):
                    if pending is not None and 4 <= qi < 4 + HEADS:
                        if qi == 4:
                            acc = accp.tile([128, DIM], f32, name="acc")
                        proj_head(qi - 4, pending[1], pending[2], acc)
                    if kind == "k":
                        kquad(kvt, kt, a0, a1)
                    else:
                        vquad(kvt, vt, a0)
                    if pending is not None and qi == 4 + HEADS:
                        nc.sync.dma_start(out=out_d[pending[0]], in_=acc)
                        pending = None

                # ---- B phase: attention; prefill next batch's kvt + kt(0,1)
                # ---- in the ACT-paced slot gaps.
                if not last:
                    kv_next = fetch_kv(steps[i + 1])
                    kt_next = [
                        ktp.tile([128, N], bf16, name="k_t") for _ in range(2)
                    ]
                    vt_next = [
                        vp.tile([128, HEADS, 65], bf16, name="v_t")
                        for _ in range(VPREF)
                    ]
                    kq = [("k", ko, ch) for ko in (0, 1) for ch in range(8)]
                    kq += [("v", tt, 0) for tt in range(VPREF)]
                else:
                    kv_next, kt_next, vt_next, kq = None, None, None, []
                xaug = xaps.tile([65, HEADS, 128], f32, name="xaug")
                prev = None  # (pr, pt0, pt1)
                slot = 0
                for pr in range(4):
                    pt0 = ptp.tile([128, N], bf16, name="pt0")
                    pt1 = ptp.tile([128, N], bf16, name="pt1")
                    for g in range(NG):
                        ps0 = qkps.tile([128, GW], f32, name="ps_s0")
                        ps1 = qkps.tile([128, GW], f32, name="ps_s1")
                        for j in range(GW // 128):
                            t = (GW // 128) * g + j
                            if "qk" in SKIP and j > 0:
                                continue
                            nc.tensor.matmul(
                                ps0[:, j * 128:(j + 1) * 128],
                                kt[pr][0:64, t * 128:(t + 1) * 128],
                                qhT[pr][0:64, b * QN:(b + 1) * QN],
                                start=True,
                                stop=True,
                                tile_position=(0, 0),
                            )
                            nc.tensor.matmul(
                                ps1[:, j * 128:(j + 1) * 128],
                                kt[pr][64:128, t * 128:(t + 1) * 128],
                                qhT[pr][64:128, b * QN:(b + 1) * QN],
                                start=True,
                                stop=True,
                                tile_position=(64, 0),
                            )
                        sl = slice(g * GW, (g + 1) * GW)
                        if "exp" not in SKIP:
                            nc.scalar.activation(pt0[:, sl], ps0, EXP)
                            nc.scalar.activation(pt1[:, sl], ps1, EXP)
                        else:
                            nc.scalar.activation(pt0[:, g * GW:g * GW + 128], ps0[:, 0:128], EXP)
                            nc.scalar.activation(pt1[:, g * GW:g * GW + 128], ps1[:, 0:128], EXP)
                        if "emmul" not in SKIP:
                            nc.vector.tensor_mul(pt0[:, sl], pt0[:, sl], em_t[:, sl])
                            nc.vector.tensor_mul(pt1[:, sl], pt1[:, sl], em_t[:, sl])
                        if pr >= 1:
                            # AV quarter of pair pr-1 (serial per head)
                            h = 2 * (pr - 1) + g // 2
                            pt = prev[1] if g < 2 else prev[2]
                            t0 = (g % 2) * (NT // 2)
                            emit_av_chunk(xaug, vt, h, pt, range(t0, t0 + NT // 2))
                            for _ in range(3 if slot >= 8 else 2):
                                if kq:
                                    kind2, a20, a21 = kq.pop(0)
                                    if kind2 == "k":
                                        kquad(kv_next, kt_next, a20, a21)
                                    else:
                                        vquad(kv_next, vt_next, a20)
                            slot += 1
                    if dbg and i == 0 and pr == 0:
                        nc.sync.dma_start(out=dbg_pt[0], in_=pt0)
                        nc.sync.dma_start(out=dbg_pt[1], in_=pt1)
                    prev = (pr, pt0, pt1)
                if dbg and i == 0:
                    for t in range(4):
                        nc.sync.dma_start(out=dbg_kt[t], in_=kt[t])
                    for t in range(2):
                        nc.sync.dma_start(out=dbg_v[t], in_=vt[t])
                av_carry = (xaug, vt, prev[1], prev[2], b)
                if not last:
                    kvt = kv_next
                    em_t = fetch_em(steps[i + 1])

            # epilogue: flush last batch
            xaug_p, vt_p, pt0_p, pt1_p, b_p = av_carry
            emit_av_pair(xaug_p, vt_p, 3, pt0_p, pt1_p)
            x_sb, recips = proj_denoms(xaug_p)
            acc = accp.tile([128, DIM], f32, name="acc")
            for h in range(HEADS):
                proj_head(h, x_sb, recips, acc)
            nc.sync.dma_start(out=out_d[b_p], in_=acc)
    return nc


def build(reps=1):
    if reps not in _built:
        nc = bacc.Bacc(
            "TRN2", target_bir_lowering=False, debug=False, num_devices=NCORES
        )
        _emit(nc, reps)
        nc.compile()
        _built[reps] = nc
    return _built[reps]


def prep_inputs(q, kv, key_mask, Wq, Wkv, Wproj, bproj):
    """Host-side shard + layout prep. Returns per-core in_maps."""
    q = np.asarray(q, dtype=np.float32)
    kv = np.asarray(kv, dtype=np.float32)
    key_mask = np.asarray(key_mask, dtype=np.float32)
    wkvT_f = np.ascontiguousarray(np.asarray(Wkv, np.float32).T)  # [512, 1024]
    if FP8 == "kv":
        # c = (half*2 + i)*128 + k  ->  wkv8[half][k, i, :]
        wkv8 = (wkvT_f * W8SCALE).reshape(2, 2, 128, 2 * DIM)
        wkv8 = np.ascontiguousarray(wkv8.transpose(0, 2, 1, 3)).astype(F8)
    else:
        wkvT = wkvT_f.astype(BF).reshape(4, 128, 2 * DIM)
    wqT = np.ascontiguousarray((np.asarray(Wq, np.float32) * SCALE).T).astype(BF)
    wqT = wqT.reshape(4, 128, DIM)
    wpT = np.ascontiguousarray(np.asarray(Wproj, np.float32).T).astype(BF)
    wpT = wpT.reshape(HEADS, 64, DIM)
    biasb = np.ascontiguousarray(
        np.broadcast_to(np.asarray(bproj, np.float32), (128, DIM))
    )

    kv_bf = kv.astype(BF)
    em = np.exp(key_mask).astype(BF)  # [B, QN, N]

    in_maps = []
    for c in range(NCORES):
        sl = slice(c * BL, (c + 1) * BL)
        q_loc = q[sl].astype(BF)  # [BL, QN, DIM]
        qT = np.ascontiguousarray(q_loc.transpose(2, 0, 1)).reshape(4, 128, BL * QN)
        # emT[b, n_in_tile, tt*128 + q] = em[b, q, tt*128 + n_in_tile]
        emT = em[sl].reshape(BL, QN, NT, 128).transpose(0, 3, 2, 1)
        emT = np.ascontiguousarray(emT).reshape(BL, 128, N)
        m = {"qT": qT, "emT": emT, "wqT": wqT, "wpT": wpT, "biasb": biasb}
        if FP8 == "kv":
            kvc = kv[sl].transpose(0, 2, 1)  # [BL, 512, N] f32
            kv8 = kvc.reshape(BL, 2, 2, 128, N).transpose(0, 1, 3, 2, 4)
            m["kvT8"] = np.ascontiguousarray(kv8).astype(F8)
            m["wkv8"] = wkv8
        else:
            m["kvT"] = np.ascontiguousarray(kv_bf[sl].transpose(0, 2, 1)).reshape(
                BL, 4, 128, N
            )
            m["wkvT"] = wkvT
        in_maps.append(m)
    return in_maps


class Runner:
    """Jitted SPMD executor with device-resident inputs for repeat timing."""

    def __init__(self, reps=1):
        import jax
        from concourse.bass2jax import (
            _bass_exec_p,
            fast_dispatch_compile,
            install_neuronx_cc_hook,
            partition_id_tensor,
        )
        from jax.experimental.shard_map import shard_map
        from jax.sharding import Mesh, PartitionSpec

        self.jax = jax
        nc = build(reps)
        install_neuronx_cc_hook()
        pname = nc.partition_id_tensor.name if nc.partition_id_tensor else None
        in_names, out_names, out_avals, in_avals = [], [], [], []
        for alloc in nc.m.functions[0].allocations:
            if not isinstance(alloc, mybir.MemoryLocationSet):
                continue
            name = alloc.memorylocations[0].name
            if alloc.kind == "ExternalInput":
                if name != pname:
                    in_names.append(name)
                    in_avals.append(
                        jax.core.ShapedArray(
                            tuple(alloc.tensor_shape), mybir.dt.np(alloc.dtype)
                        )
                    )
            elif alloc.kind == "ExternalOutput":
                out_names.append(name)
                out_avals.append(
                    jax.core.ShapedArray(
                        tuple(alloc.tensor_shape), mybir.dt.np(alloc.dtype)
                    )
                )
        self.in_names = list(in_names)
        self.in_avals = in_avals
        self.out_names = out_names
        self.out_avals = out_avals
        n_params = len(in_names)
        all_names = in_names + out_names
        if pname is not None:
            all_names = all_names + [pname]
        donate = tuple(range(n_params, n_params + len(out_names)))

        def _body(*args):
            operands = list(args)
            if pname is not None:
                operands.append(partition_id_tensor())
            outs = _bass_exec_p.bind(
                *operands,
                out_avals=tuple(out_avals),
                in_names=tuple(all_names),
                out_names=tuple(out_names),
                lowering_input_output_aliases=(),
                sim_require_finite=True,
                sim_require_nnan=True,
                nc=nc,
            )
            return tuple(outs)

        devices = jax.devices()[:NCORES]
        self.mesh = Mesh(np.asarray(devices), ("core",))
        self.pspec = PartitionSpec("core")
        in_specs = (self.pspec,) * (n_params + len(out_names))
        out_specs = (self.pspec,) * len(out_names)
        arg_structs = [
            jax.ShapeDtypeStruct((NCORES * a.shape[0], *a.shape[1:]), a.dtype)
            for a in in_avals + out_avals
        ]

        def _compile():
            return jax.jit(
                shard_map(
                    _body,
                    mesh=self.mesh,
                    in_specs=in_specs,
                    out_specs=out_specs,
                    check_rep=False,
                ),
                donate_argnums=donate,
                keep_unused=True,
            ).lower(*arg_structs).compile()

        try:
            self.fn = fast_dispatch_compile(_compile)
        except Exception:
            self.fn = _compile()

    def put_inputs(self, in_maps):
        """Concat per-core inputs on axis 0 and move to devices (sharded)."""
        from jax.sharding import NamedSharding

        sh = NamedSharding(self.mesh, self.pspec)
        dev = []
        for name, aval in zip(self.in_names, self.in_avals):
            cat = np.concatenate([m[name] for m in in_maps], axis=0)
            if cat.dtype != aval.dtype:
                cat = cat.view(aval.dtype)  # same bits (fp8 variant naming)
            dev.append(self.jax.device_put(cat, sh))
        return dev

    def zeros(self):
        from jax.sharding import NamedSharding

        sh = NamedSharding(self.mesh, self.pspec)
        return [
            self.jax.device_put(
                np.zeros((NCORES * a.shape[0], *a.shape[1:]), a.dtype), sh
            )
            for a in self.out_avals
        ]

    def run(self, dev_inputs, zeros=None):
        if zeros is None:
            zeros = self.zeros()
        outs = self.fn(*dev_inputs, *zeros)
        self.jax.block_until_ready(outs)
        return outs


def get_runner(reps=1):
    if reps not in _runner:
        _runner[reps] = Runner(reps)
    return _runner[reps]


def kernel(q, kv, key_mask, Wq, Wkv, Wproj, bproj):
    r = get_runner()
    in_maps = prep_inputs(q, kv, key_mask, Wq, Wkv, Wproj, bproj)
    dev = r.put_inputs(in_maps)
    outs = r.run(dev)
    out = np.asarray(outs[0]).reshape(NCORES, BL, QN, DIM).reshape(B, QN, DIM)
    return out.astype(np.float32)

